# revision 23
# baseline (speedup 1.0000x reference)
"""Tensor-parallel causal attention block for Trainium2 (8 NeuronCores).

Sharding: tensor-parallel across heads (2 heads/core) for QKV+attention,
then one AllToAll per local head (fp16, 2MB) to switch to row-parallel
for the output projection. Measured HW exec ~727us vs 975us baseline.

Structure:
- Head-major attention passes: pass A runs proj(b)+attention(b, h=0) for
  all batches with the NEXT batch's projection matmuls interleaved into
  the attention emission (fills the PE bubbles left by Act-engine exp
  pacing); AllToAll(h0) then overlaps the whole h=1 pass; phase 3 leads
  with h0 accumulation (8-pout stagger) to cover AllToAll(h1).
- PV matmuls use exp(S^T) 128-col sub-blocks as the STATIONARY operand
  and [V | ones] (129 cols) as the moving operand: the softmax
  denominator falls out of the same matmul (column 128), eliminating the
  separate ones-row denominator matmuls and the reciprocal-broadcast
  matmul. PV output is naturally [q, d]; normalization is a
  per-partition tensor_scalar multiply; staging is written [n, d]-major
  (contiguous per A2A shard) and re-transposed after the collective with
  the DMA transpose XBAR.
- PSUM accumulation groups sharing a bank must not interleave their
  start=True flags (first_mm clears the whole bank's has_written bits):
  only the first matmul into each po pair-tile sets start.
- Exact-causal trimming: diagonal score matmuls slice off fully-masked
  columns, PV skips fully-masked sub-blocks, the second diagonal exp
  splits into live ranges, and the causal mask is a single 128x128
  triangular tile applied only to the diagonal sub-block.
- RoPE is fused into the QKV PSUM eviction as 4 full-partition DVE ops
  using host-built [cos;cos] / [-sin;sin] fp16 tables loaded once.
- Bulk single-instruction DMAs (weights, x, wo, yt) cut SWDGE
  descriptor-generation cost; wo rides the Act HWDGE queue deferred
  behind pass-A staging; yt transpose-loads are dep-pinned behind pass-B
  staging so the greedy list scheduler cannot head-of-line-block the SP
  DMA queue against an in-flight collective.
- v-eviction runs on DVE so the Act engine's activation table stays on
  Exp (each Copy<->Exp switch costs a ~1.3us table load).

All matmul inputs fp16; accumulation fp32 in PSUM; output fp32.
"""
from functools import partial

import numpy as np

import concourse.bass as bass
import concourse.tile as tile
import concourse.mybir as mybir
from concourse.bass_utils import run_bass_kernel_spmd
from concourse.tile_rust import add_dep_helper

N_CORES = 8
B, T, C = 4, 2048, 2048
H = 16                 # total heads
HPC = H // N_CORES     # heads per core = 2
D = C // H             # head dim = 128
HALF = D // 2
P = 128                # partitions
TG = 512               # attention query group
NTG = T // TG          # 4
NCC = C // P           # 16 contraction chunks
NSLICE = B * T // N_CORES  # 1024 output rows per core
TGP = 1024             # projection t-group
XCH = 512              # xt chunk columns
VW = D + 1             # 129: V plus fused ones column

FP = mybir.dt.float32
FP16 = mybir.dt.float16
EXP = mybir.ActivationFunctionType.Exp
SCALE = 1.0 / float(np.sqrt(D))

# ---------------------------------------------------------------------------
# Workaround: this container's walrus rejects >1 sync-wait per instruction.
# Hoist extras onto preceding same-engine NoOps (engine streams are in-order).
# ---------------------------------------------------------------------------
from concourse.vector_clock import ScopedClock


def _fixup_multiwaits(nc):
    moved = 0
    for fn in nc.m.functions:
        for bb in fn.blocks:
            insts = bb.instructions
            if not any(
                i.sync_info and i.sync_info.on_wait and len(i.sync_info.on_wait) > 1
                for i in insts
            ):
                continue
            new_insts = []
            for ins in insts:
                si = ins.sync_info
                if si is not None and si.on_wait and len(si.on_wait) > 1:
                    extra, keep = si.on_wait[:-1], si.on_wait[-1:]
                    for w in extra:
                        nop = mybir.InstNoOp(
                            name=nc.get_next_instruction_name(),
                            ins=[],
                            outs=[],
                            engine=ins.engine,
                        )
                        nop.sync_info = mybir.SyncInfo(on_wait=[w], on_update=[])
                        new_insts.append(nop)
                        moved += 1
                    si.on_wait = keep
                new_insts.append(ins)
            bb.instructions = new_insts
    return moved


def _patched_drain_and_barrier(self, tick_clock, wait_clock):
    nop = self.nc.sync.nop(nofuse=True)
    wait_clock.add_sem_waits(nop.ins, ScopedClock({None: tick_clock.global_clock}))
    w = nop.ins.sync_info.on_wait if nop.ins.sync_info else []
    while w and len(w) > 1:
        cond = w.pop()
        n2 = self.nc.sync.nop(nofuse=True)
        if n2.ins.sync_info is None:
            n2.ins.sync_info = mybir.SyncInfo(on_wait=[], on_update=[])
        n2.ins.sync_info.on_wait.append(cond)
    self.nc.sync.drain()
    self.nc.all_engine_barrier()
    assert self.sems is not None
    popped = self.nc._tile_sem_poison_stack.pop()
    assert popped is self._sem_poison
    self.nc.clear_and_free_semaphores(list(self.sems.allocated().values()))
    self.nc.all_engine_barrier()


tile.TileContext._drain_and_barrier = _patched_drain_and_barrier

# SBUF cap: tile_utils caps at 192KB/partition; cayman has 208 usable.
try:
    import concourse.tile_utils as _tile_utils

    if getattr(_tile_utils, "max_sbuf_usage", None) is not None:
        _tile_utils.max_sbuf_usage = 204 * 1024
except Exception:
    pass


# ---------------------------------------------------------------------------
# Device program
# ---------------------------------------------------------------------------
def build_program(reps: int = 1, mode: str = "full"):
    nc = bass.Bass()

    xT = nc.dram_tensor("xT", [B, C, T], FP16, kind="ExternalInput")
    wqT = nc.dram_tensor("wqT", [C, HPC * D], FP16, kind="ExternalInput")
    wkT = nc.dram_tensor("wkT", [C, HPC * D], FP16, kind="ExternalInput")
    wvT = nc.dram_tensor("wvT", [C, HPC * D], FP16, kind="ExternalInput")
    woT = nc.dram_tensor("woT", [C, C], FP16, kind="ExternalInput")
    cosC = nc.dram_tensor("cosC", [P, T], FP16, kind="ExternalInput")
    sinS = nc.dram_tensor("sinS", [P, T], FP16, kind="ExternalInput")
    maskd = nc.dram_tensor("maskd", [P, P], FP16, kind="ExternalInput")

    out_rows = nc.dram_tensor("out_rows", [NSLICE, C], FP, kind="ExternalOutput")

    with tile.TileContext(nc) as tc:
        with tc.tile_pool(name="const", bufs=1) as const:
            mask_s = const.tile([P, P], FP16, name="mask_s")
            wq_all = const.tile([P, NCC * HPC * D], FP16, name="wq_all")
            wk_all = const.tile([P, NCC * HPC * D], FP16, name="wk_all")
            wv_all = const.tile([P, NCC * HPC * D], FP16, name="wv_all")
            cos_s = const.tile([P, T], FP16, name="cos_s")
            sin_s = const.tile([P, T], FP16, name="sin_s")
            consts = dict(
                mask_s=mask_s, wq_all=wq_all, wk_all=wk_all, wv_all=wv_all,
                cos_s=cos_s, sin_s=sin_s,
                wqT=wqT, wkT=wkT, wvT=wvT, cosC=cosC, sinS=sinS, maskd=maskd,
            )
            for rep in range(reps):
                _emit_body(nc, tc, rep, xT, woT, out_rows, consts, mode=mode)

    moved = _fixup_multiwaits(nc)
    return nc, moved


def _load_weight(nc, dst, src):
    # src [C, W] DRAM row-major -> dst [P, NCC, W] (partition p = row%128)
    nc.sync.dma_start(
        dst[:].rearrange("p (cc w) -> p cc w", cc=NCC),
        src[:, :].rearrange("(cc p) w -> p cc w", p=P),
    )


def _emit_body(nc, tc, rep, xT, woT, out_rows, cst, mode="full"):
    ya_in = [
        nc.dram_tensor(f"ya_in_{rep}_{h}", [B * T, D], FP16) for h in range(HPC)
    ]
    ya_out = [
        nc.dram_tensor(f"ya_out_{rep}_{h}", [B * T, D], FP16) for h in range(HPC)
    ]

    ab = tc.alloc_tile_pool(name=f"ab{rep}", bufs=1)
    ps = tc.alloc_tile_pool(name=f"ps{rep}", bufs=1, space="PSUM")
    pa = tc.alloc_tile_pool(name=f"pa{rep}", bufs=1)

    _pending_w = [None]
    w_all = {"q": cst["wq_all"], "k": cst["wk_all"]}
    qk = {}
    vall = {}

    def load_xt(b, tg):
        """Four chunk tiles of 4 cc each covering the tg's 1024 cols. The
        first call also flushes the deferred mask/wq loads right after its
        first group (startup is DMA-arrival-paced; x goes first)."""
        out = []
        for grp in range(4):
            t = pa.tile(
                [P, 4 * TGP], FP16, tag=f"xt{grp}", bufs=2,
                name=f"xt{grp}_{rep}_{b}_{tg}",
            )
            nc.sync.dma_start(
                t[:].rearrange("p (cc w) -> p cc w", cc=4),
                xT[b][
                    P * 4 * grp : P * 4 * (grp + 1), TGP * tg : TGP * (tg + 1)
                ].rearrange("(cc p) w -> p cc w", p=P),
            )
            if _pending_w[0] is not None:
                _pending_w[0]()
                _pending_w[0] = None
            out.append(t)
        return out

    def xsl(xts, cc, c0, w):
        """Moving slice of x chunk cc covering cols [c0, c0+w) of the tg group."""
        t = xts[cc // 4]
        cc0 = cc % 4
        return t[:, TGP * cc0 + c0 : TGP * cc0 + c0 + w]

    def proj_thunks(b):
        """Projection for batch b as a list of thunks, so pass-A attention
        can interleave them into its emission (fills Act-paced PE bubbles)."""
        state = {}
        thunks = []

        def t_alloc():
            va = ab.tile(
                [P, NCC * 2 * VW], FP16, tag="vall", bufs=4, name=f"va_{rep}_{b}"
            )
            vall[b] = va
            nc.gpsimd.memset(
                va[:].rearrange("p (t x) -> p t x", x=VW)[:, :, D : D + 1], 1.0
            )
            for pj in ("q", "k"):
                for h in range(HPC):
                    pool = pa if h == 0 else ab
                    qk[(pj, h, b)] = pool.tile(
                        [P, T], FP16, tag=f"{pj}T{h}", bufs=(2 if h == 0 else 4),
                        name=f"{pj}T{h}_{rep}_{b}",
                    )
            state[0] = load_xt(b, 0)

        thunks.append(t_alloc)

        def t_xt1():
            state[1] = load_xt(b, 1)

        def t_pmm(tg, pj, h):
            xts = state[tg]
            pmm = ps.tile(
                [P, TGP], FP, tag="big2", bufs=3, name=f"pmm_{rep}_{b}_{tg}_{pj}{h}"
            )
            # cc-outer: both 512-halves run back-to-back on the same
            # stationary, halving distinct LDWEIGHTS loads
            for cc in range(NCC):
                for half in range(2):
                    nc.tensor.matmul(
                        pmm[:, XCH * half : XCH * (half + 1)],
                        w_all[pj][:, (HPC * D) * cc + D * h : (HPC * D) * cc + D * (h + 1)],
                        xsl(xts, cc, XCH * half, XCH),
                        start=(cc == 0),
                        stop=(cc == NCC - 1),
                    )
            # RoPE eviction: dst = pmm*C + rot(pmm)*S
            tc_sl = cst["cos_s"][:, TGP * tg : TGP * (tg + 1)]
            ts_sl = cst["sin_s"][:, TGP * tg : TGP * (tg + 1)]
            t1 = pa.tile([P, TGP], FP, tag="t1", bufs=2, name=f"t1_{rep}_{b}_{tg}_{pj}{h}")
            t2 = pa.tile([P, TGP], FP, tag="t2", bufs=2, name=f"t2_{rep}_{b}_{tg}_{pj}{h}")
            nc.vector.tensor_mul(t1[:], pmm[:], tc_sl)
            nc.vector.tensor_mul(t2[0:HALF, :], pmm[HALF:P, :], ts_sl[0:HALF, :])
            nc.vector.tensor_mul(t2[HALF:P, :], pmm[0:HALF, :], ts_sl[HALF:P, :])
            nc.vector.tensor_add(
                qk[(pj, h, b)][:, TGP * tg : TGP * (tg + 1)], t1[:], t2[:]
            )

        def t_v(tg, vg):
            xts = state[tg]
            pv = ps.tile([P, TGP], FP, tag="big2", bufs=3, name=f"pv_{rep}_{b}_{tg}_{vg}")
            for ts4 in range(4):
                for cc in range(NCC):
                    nc.tensor.matmul(
                        pv[:, (HPC * D) * ts4 : (HPC * D) * (ts4 + 1)],
                        xsl(xts, cc, XCH * vg + P * ts4, P),
                        cst["wv_all"][:, (HPC * D) * cc : (HPC * D) * (cc + 1)],
                        start=(cc == 0),
                        stop=(cc == NCC - 1),
                    )
            tch0 = 8 * tg + 4 * vg
            # DVE, not Act: keeps the Act engine's activation table on Exp
            # (each Copy<->Exp switch costs a ~1.3us table load)
            nc.vector.tensor_copy(
                vall[b][:, 2 * VW * tch0 : 2 * VW * (tch0 + 4)].rearrange(
                    "p (t h d) -> p t h d", h=HPC, d=VW
                )[:, :, :, 0:D],
                pv[:].rearrange("p (t h d) -> p t h d", h=HPC, d=D),
            )

        for tg in range(T // TGP):
            if tg == 1:
                thunks.append(t_xt1)
            for pj, h in (("q", 0), ("k", 0), ("q", 1), ("k", 1)):
                thunks.append(partial(t_pmm, tg, pj, h))
            for vg in range(2):
                thunks.append(partial(t_v, tg, vg))
        return thunks

    last_staging = [None]

    def attn(b, h, filler=()):
        filler = list(filler)
        qT = qk[("q", h, b)]
        kT = qk[("k", h, b)]
        va = vall[b]
        seq = [(g, pi) for g in range(NTG) for pi in range(2 * (g + 1))]
        po = {}

        def emit_pv(g, pi, pt):
            poA, poB = po[g]
            for half in range(2):
                i = 2 * pi + half
                jj = i - 4 * g
                for s in range(4):
                    if jj >= 0 and s < jj:
                        continue
                    pair, slot = divmod(s, 2)
                    dst = poA if pair == 0 else poB
                    # start=True clears the whole bank's has_written bits, so
                    # only the FIRST matmul into each pair-tile may set it —
                    # slot 1's first write relies on has_written=0 to store.
                    nc.tensor.matmul(
                        dst[:, VW * slot : VW * (slot + 1)],
                        pt[:, TG * half + P * s : TG * half + P * (s + 1)],
                        va[:, 2 * VW * i + VW * h : 2 * VW * i + VW * (h + 1)],
                        start=(i == 0 and slot == 0),
                        stop=(i == 4 * g + s),
                    )
            if pi == 2 * (g + 1) - 1:
                # group done: normalize + stage
                for pair in range(2):
                    pp = po[g][pair]
                    rc = ab.tile([P, 2], FP, tag="rc", bufs=4, name=f"rc_{rep}_{b}_{h}_{g}_{pair}")
                    ytn = ab.tile(
                        [P, 2 * D], FP16, tag=f"ytn{pair}", bufs=4,
                        name=f"ytn_{rep}_{b}_{h}_{g}_{pair}",
                    )
                    for slot in range(2):
                        nc.vector.reciprocal(
                            rc[:, slot : slot + 1], pp[:, VW * slot + D : VW * slot + D + 1]
                        )
                        nc.vector.tensor_scalar_mul(
                            ytn[:, D * slot : D * (slot + 1)],
                            pp[:, VW * slot : VW * slot + D],
                            rc[:, slot : slot + 1],
                        )
                    n0 = T * b + TG * g + 2 * P * pair
                    last_staging[0] = nc.sync.dma_start(
                        ya_in[h][n0 : n0 + 2 * P, :].rearrange("(s p) d -> p s d", s=2),
                        ytn[:].rearrange("p (s d) -> p s d", s=2),
                    )

        pending = None
        for g, pi in seq:
            if pi == 0:
                po[g] = (
                    ps.tile([P, 2 * VW], FP, tag="poA", bufs=1, name=f"poA_{rep}_{b}_{h}_{g}"),
                    ps.tile([P, 2 * VW], FP, tag="poB", bufs=1, name=f"poB_{rep}_{b}_{h}_{g}"),
                )
            pss = ps.tile([P, 2 * TG], FP, tag="big2", bufs=3, name=f"pss_{rep}_{b}_{h}_{g}_{pi}")
            for half in range(2):
                i = 2 * pi + half
                jj = i - 4 * g
                r = P * jj if jj >= 0 else 0
                nc.tensor.matmul(
                    pss[:, TG * half + r : TG * (half + 1)],
                    kT[:, P * i : P * (i + 1)],
                    qT[:, TG * g + r : TG * (g + 1)],
                    start=True,
                    stop=True,
                )
            pt = ab.tile([P, 2 * TG], FP16, tag="pt", bufs=3, name=f"pt_{rep}_{b}_{h}_{g}_{pi}")
            j0 = 2 * pi - 4 * g
            if j0 >= 2:
                # second diagonal pair: 640 of 1024 columns are dead-masked;
                # exp only the live ranges (two Act instrs beat 640 wasted cols)
                for half in range(2):
                    r = P * (j0 + half)
                    sl_ = slice(TG * half + r, TG * (half + 1))
                    nc.scalar.activation(pt[:, sl_], pss[:, sl_], EXP, scale=SCALE)
            else:
                nc.scalar.activation(pt[:], pss[:], EXP, scale=SCALE)
            for half in range(2):
                i = 2 * pi + half
                jj = i - 4 * g
                if jj >= 0:
                    sl_ = slice(TG * half + P * jj, TG * half + P * (jj + 1))
                    nc.vector.tensor_mul(pt[:, sl_], pt[:, sl_], cst["mask_s"][:])
            if pending is not None:
                emit_pv(*pending)
            if filler:
                filler.pop(0)()
            pending = (g, pi, pt)
        emit_pv(*pending)
        for t in filler:
            t()

    # ---- pass A: projections + h0 attention ---------------------------
    first = rep == 0
    if first:
        # SP queue: first xt chunk group goes ahead of the weight loads
        # (the first matmul chain is DMA-arrival-paced); Act HWDGE queue
        # issues wk/cos/sin/wv in parallel.
        _pending_w[0] = lambda: (
            nc.sync.dma_start(cst["mask_s"][:], cst["maskd"][:]),
            _load_weight(nc, cst["wq_all"], cst["wqT"]),
        )
        nc.scalar.dma_start(
            cst["wk_all"][:].rearrange("p (cc w) -> p cc w", cc=NCC),
            cst["wkT"][:, :].rearrange("(cc p) w -> p cc w", p=P),
        )
        nc.scalar.dma_start(cst["cos_s"][:], cst["cosC"][:])
        nc.scalar.dma_start(cst["sin_s"][:], cst["sinS"][:])
        nc.scalar.dma_start(
            cst["wv_all"][:].rearrange("p (cc w) -> p cc w", cc=NCC),
            cst["wvT"][:, :].rearrange("(cc p) w -> p cc w", p=P),
        )
    for t in proj_thunks(0):
        t()
    for b in range(B):
        filler = proj_thunks(b + 1) if b + 1 < B else []
        attn(b, 0, filler)

    if mode == "full":
        nc.gpsimd.collective_compute(
            "AllToAll",
            mybir.AluOpType.bypass,
            replica_groups=[list(range(N_CORES))],
            ins=[ya_in[0][:]],
            outs=[ya_out[0][:]],
        )

    pa.release()
    wop = tc.alloc_tile_pool(name=f"wo{rep}", bufs=1, side="right")
    ytp = tc.alloc_tile_pool(name=f"ytp{rep}", bufs=1, side="right")
    op3 = tc.alloc_tile_pool(name=f"op3{rep}", bufs=1, side="right")

    # wo load rides the Activation HWDGE queue (SP would head-of-line-block
    # pass-B staging writes behind an 8MB transfer) and is held back until
    # pass-A staging is out, so it doesn't steal startup DMA bandwidth.
    wo_all = wop.tile([P, NCC * C], FP16, tag="wo", name=f"wo_{rep}")
    for wg in range(4):
        wo_dma = nc.scalar.dma_start(
            wo_all[:, NCC * C // 4 * wg : NCC * C // 4 * (wg + 1)].rearrange(
                "p (cc w) -> p cc w", cc=NCC // 4
            ),
            woT[C // 4 * wg : C // 4 * (wg + 1), :].rearrange(
                "(cc p) w -> p cc w", p=P
            ),
        )
        add_dep_helper(wo_dma.ins, last_staging[0].ins, reason="defer wo load")
    yt_all = {
        h: ytp.tile([P, N_CORES * NSLICE], FP16, tag=f"yt{h}", name=f"yt{h}_{rep}")
        for h in range(HPC)
    }

    def load_yt(h, after=None):
        for j in range(N_CORES):
            t = nc.sync.dma_start(
                yt_all[h][:, NSLICE * j : NSLICE * (j + 1)],
                ya_out[h][NSLICE * j : NSLICE * (j + 1), :],
                transpose=True,
            )
            if after is not None:
                # keep the greedy list scheduler from hoisting these into the
                # middle of pass B, where they head-of-line-block the SP DMA
                # queue (staging writes) behind the still-running AllToAll
                add_dep_helper(t.ins, after.ins, reason="defer yt load")

    # ---- pass B: h1 attention (AllToAll(h0) in flight) ----------------
    for b in range(B):
        attn(b, 1)
    # yt(h0) loads go after ALL pass-B staging writes: A2A(h0) is done by
    # now, so these fire immediately without blocking the SP queue.
    load_yt(0, after=last_staging[0])

    if mode == "full":
        nc.gpsimd.collective_compute(
            "AllToAll",
            mybir.AluOpType.bypass,
            replica_groups=[list(range(N_CORES))],
            ins=[ya_in[1][:]],
            outs=[ya_out[1][:]],
        )
    load_yt(1)

    ps.release()
    ps3 = tc.alloc_tile_pool(name=f"ps3{rep}", bufs=1, space="PSUM")

    # ---- output projection, h0-staggered ------------------------------
    jobs = [(jg, nt) for jg in range(C // TG) for nt in range(NSLICE // P)]
    pouts = {}

    def h_mms(idx, hs):
        jg, nt = jobs[idx]
        for j in range(N_CORES):
            ccg = HPC * j + hs
            nc.tensor.matmul(
                pouts[idx][:],
                yt_all[hs][:, NSLICE * j + P * nt : NSLICE * j + P * (nt + 1)],
                wo_all[:, C * ccg + TG * jg : C * ccg + TG * (jg + 1)],
                start=(hs == 0 and j == 0),
                stop=(hs == 1 and j == N_CORES - 1),
            )

    STAG = 8
    for idx in range(len(jobs) + STAG):
        if idx < len(jobs):
            pouts[idx] = ps3.tile([P, TG], FP, tag="pout", bufs=STAG, name=f"pout_{rep}_{idx}")
            h_mms(idx, 0)
        if idx >= STAG:
            k = idx - STAG
            h_mms(k, 1)
            jg, nt = jobs[k]
            ot = op3.tile([P, TG], FP, tag="ot", bufs=2, name=f"ot_{rep}_{k}")
            nc.scalar.copy(ot[:], pouts[k][:])
            nc.sync.dma_start(
                out_rows[P * nt : P * (nt + 1), TG * jg : TG * (jg + 1)], ot[:]
            )
            del pouts[k]

    ps3.release()
    op3.release()
    ytp.release()
    wop.release()
    ab.release()


# ---------------------------------------------------------------------------
# Host-side prep + execution
# ---------------------------------------------------------------------------
def _host_inputs(x, wq, wk, wv, wo):
    xT = np.ascontiguousarray(x.transpose(0, 2, 1)).astype(np.float16)
    woT = np.ascontiguousarray(wo.T).astype(np.float16)

    freqs = 1.0 / (10000.0 ** (np.arange(HALF, dtype=np.float32) / HALF))
    t = np.arange(T, dtype=np.float32)
    ang = freqs[:, None] * t[None, :]  # [64, T]
    cosC = np.concatenate([np.cos(ang), np.cos(ang)], axis=0).astype(np.float16)
    sinS = np.concatenate([-np.sin(ang), np.sin(ang)], axis=0).astype(np.float16)

    # maskd[k, q] = 1.0 iff q >= k
    maskd = np.triu(np.ones((P, P), dtype=np.float16))

    common = dict(xT=xT, woT=woT, cosC=cosC, sinS=sinS, maskd=maskd)
    in_maps = []
    for r in range(N_CORES):
        rows = slice(HPC * D * r, HPC * D * (r + 1))
        in_maps.append(
            dict(
                common,
                wqT=np.ascontiguousarray(wq[rows, :].T).astype(np.float16),
                wkT=np.ascontiguousarray(wk[rows, :].T).astype(np.float16),
                wvT=np.ascontiguousarray(wv[rows, :].T).astype(np.float16),
            )
        )
    return in_maps


_CACHED = {}


def _get_program(reps=1):
    if reps not in _CACHED:
        _CACHED[reps] = build_program(reps)[0]
    return _CACHED[reps]


def kernel(x, wq, wk, wv, wo):
    nc = _get_program(1)
    in_maps = _host_inputs(
        np.asarray(x, dtype=np.float32),
        np.asarray(wq, dtype=np.float32),
        np.asarray(wk, dtype=np.float32),
        np.asarray(wv, dtype=np.float32),
        np.asarray(wo, dtype=np.float32),
    )
    res = run_bass_kernel_spmd(nc, in_maps, list(range(N_CORES)))
    out = np.concatenate([res.results[r]["out_rows"] for r in range(N_CORES)], axis=0)
    return out.reshape(B, T, C)


# revision 24
# speedup vs baseline: 1.0177x; 1.0177x over previous
"""Tensor-parallel causal attention block for Trainium2 (8 NeuronCores).

Sharding: tensor-parallel across heads (2 heads/core) for QKV+attention,
then one AllToAll per local head (fp16, 2MB) to switch to row-parallel
for the output projection. Measured HW exec ~727us vs 975us baseline.

Structure:
- Head-major attention passes: pass A runs proj(b)+attention(b, h=0) for
  all batches with the NEXT batch's projection matmuls interleaved into
  the attention emission (fills the PE bubbles left by Act-engine exp
  pacing); AllToAll(h0) then overlaps the whole h=1 pass; phase 3 leads
  with h0 accumulation (8-pout stagger) to cover AllToAll(h1).
- PV matmuls use exp(S^T) 128-col sub-blocks as the STATIONARY operand
  and [V | ones] (129 cols) as the moving operand: the softmax
  denominator falls out of the same matmul (column 128), eliminating the
  separate ones-row denominator matmuls and the reciprocal-broadcast
  matmul. PV output is naturally [q, d]; normalization is a
  per-partition tensor_scalar multiply; staging is written [n, d]-major
  (contiguous per A2A shard) and re-transposed after the collective with
  the DMA transpose XBAR.
- PSUM accumulation groups sharing a bank must not interleave their
  start=True flags (first_mm clears the whole bank's has_written bits):
  only the first matmul into each po pair-tile sets start.
- Exact-causal trimming: diagonal score matmuls slice off fully-masked
  columns, PV skips fully-masked sub-blocks, the second diagonal exp
  splits into live ranges, and the causal mask is a single 128x128
  triangular tile applied only to the diagonal sub-block.
- RoPE is fused into the QKV PSUM eviction as 4 full-partition DVE ops
  using host-built [cos;cos] / [-sin;sin] fp16 tables loaded once.
- Bulk single-instruction DMAs (weights, x, wo, yt) cut SWDGE
  descriptor-generation cost; wo rides the Act HWDGE queue deferred
  behind pass-A staging; yt transpose-loads are dep-pinned behind pass-B
  staging so the greedy list scheduler cannot head-of-line-block the SP
  DMA queue against an in-flight collective.
- v-eviction runs on DVE so the Act engine's activation table stays on
  Exp (each Copy<->Exp switch costs a ~1.3us table load).

All matmul inputs fp16; accumulation fp32 in PSUM; output fp32.
"""
from functools import partial

import numpy as np

import concourse.bass as bass
import concourse.tile as tile
import concourse.mybir as mybir
from concourse.bass_utils import run_bass_kernel_spmd
from concourse.tile_rust import add_dep_helper

N_CORES = 8
B, T, C = 4, 2048, 2048
H = 16                 # total heads
HPC = H // N_CORES     # heads per core = 2
D = C // H             # head dim = 128
HALF = D // 2
P = 128                # partitions
TG = 512               # attention query group
NTG = T // TG          # 4
NCC = C // P           # 16 contraction chunks
NSLICE = B * T // N_CORES  # 1024 output rows per core
TGP = 1024             # projection t-group
XCH = 512              # xt chunk columns
VW = D + 1             # 129: V plus fused ones column

FP = mybir.dt.float32
FP16 = mybir.dt.float16
EXP = mybir.ActivationFunctionType.Exp
SCALE = 1.0 / float(np.sqrt(D))

# ---------------------------------------------------------------------------
# Workaround: this container's walrus rejects >1 sync-wait per instruction.
# Hoist extras onto preceding same-engine NoOps (engine streams are in-order).
# ---------------------------------------------------------------------------
from concourse.vector_clock import ScopedClock


def _fixup_multiwaits(nc):
    moved = 0
    for fn in nc.m.functions:
        for bb in fn.blocks:
            insts = bb.instructions
            if not any(
                i.sync_info and i.sync_info.on_wait and len(i.sync_info.on_wait) > 1
                for i in insts
            ):
                continue
            new_insts = []
            for ins in insts:
                si = ins.sync_info
                if si is not None and si.on_wait and len(si.on_wait) > 1:
                    extra, keep = si.on_wait[:-1], si.on_wait[-1:]
                    for w in extra:
                        nop = mybir.InstNoOp(
                            name=nc.get_next_instruction_name(),
                            ins=[],
                            outs=[],
                            engine=ins.engine,
                        )
                        nop.sync_info = mybir.SyncInfo(on_wait=[w], on_update=[])
                        new_insts.append(nop)
                        moved += 1
                    si.on_wait = keep
                new_insts.append(ins)
            bb.instructions = new_insts
    return moved


def _patched_drain_and_barrier(self, tick_clock, wait_clock):
    nop = self.nc.sync.nop(nofuse=True)
    wait_clock.add_sem_waits(nop.ins, ScopedClock({None: tick_clock.global_clock}))
    w = nop.ins.sync_info.on_wait if nop.ins.sync_info else []
    while w and len(w) > 1:
        cond = w.pop()
        n2 = self.nc.sync.nop(nofuse=True)
        if n2.ins.sync_info is None:
            n2.ins.sync_info = mybir.SyncInfo(on_wait=[], on_update=[])
        n2.ins.sync_info.on_wait.append(cond)
    self.nc.sync.drain()
    self.nc.all_engine_barrier()
    assert self.sems is not None
    popped = self.nc._tile_sem_poison_stack.pop()
    assert popped is self._sem_poison
    self.nc.clear_and_free_semaphores(list(self.sems.allocated().values()))
    self.nc.all_engine_barrier()


tile.TileContext._drain_and_barrier = _patched_drain_and_barrier

# SBUF cap: tile_utils caps at 192KB/partition; cayman has 208 usable.
try:
    import concourse.tile_utils as _tile_utils

    if getattr(_tile_utils, "max_sbuf_usage", None) is not None:
        _tile_utils.max_sbuf_usage = 204 * 1024
except Exception:
    pass


# ---------------------------------------------------------------------------
# Device program
# ---------------------------------------------------------------------------
def build_program(reps: int = 1, mode: str = "full"):
    nc = bass.Bass()

    xT = nc.dram_tensor("xT", [B, C, T], FP16, kind="ExternalInput")
    wqT = nc.dram_tensor("wqT", [C, HPC * D], FP16, kind="ExternalInput")
    wkT = nc.dram_tensor("wkT", [C, HPC * D], FP16, kind="ExternalInput")
    wvT = nc.dram_tensor("wvT", [C, HPC * D], FP16, kind="ExternalInput")
    woT = nc.dram_tensor("woT", [C, C], FP16, kind="ExternalInput")
    cosC = nc.dram_tensor("cosC", [P, T], FP16, kind="ExternalInput")
    sinS = nc.dram_tensor("sinS", [P, T], FP16, kind="ExternalInput")
    maskd = nc.dram_tensor("maskd", [P, P], FP16, kind="ExternalInput")

    out_rows = nc.dram_tensor("out_rows", [NSLICE, C], FP, kind="ExternalOutput")

    with tile.TileContext(nc) as tc:
        with tc.tile_pool(name="const", bufs=1) as const:
            mask_s = const.tile([P, P], FP16, name="mask_s")
            wq_all = const.tile([P, NCC * HPC * D], FP16, name="wq_all")
            wk_all = const.tile([P, NCC * HPC * D], FP16, name="wk_all")
            wv_all = const.tile([P, NCC * HPC * D], FP16, name="wv_all")
            cos_s = const.tile([P, T], FP16, name="cos_s")
            sin_s = const.tile([P, T], FP16, name="sin_s")
            consts = dict(
                mask_s=mask_s, wq_all=wq_all, wk_all=wk_all, wv_all=wv_all,
                cos_s=cos_s, sin_s=sin_s,
                wqT=wqT, wkT=wkT, wvT=wvT, cosC=cosC, sinS=sinS, maskd=maskd,
            )
            for rep in range(reps):
                _emit_body(nc, tc, rep, xT, woT, out_rows, consts, mode=mode)

    moved = _fixup_multiwaits(nc)
    return nc, moved


def _load_weight(nc, dst, src):
    # src [C, W] DRAM row-major -> dst [P, NCC, W] (partition p = row%128)
    nc.sync.dma_start(
        dst[:].rearrange("p (cc w) -> p cc w", cc=NCC),
        src[:, :].rearrange("(cc p) w -> p cc w", p=P),
    )


def _emit_body(nc, tc, rep, xT, woT, out_rows, cst, mode="full"):
    ya_in = [
        nc.dram_tensor(f"ya_in_{rep}_{h}", [B * T, D], FP16) for h in range(HPC)
    ]
    ya_out = [
        nc.dram_tensor(f"ya_out_{rep}_{h}", [B * T, D], FP16) for h in range(HPC)
    ]

    ab = tc.alloc_tile_pool(name=f"ab{rep}", bufs=1)
    ps = tc.alloc_tile_pool(name=f"ps{rep}", bufs=1, space="PSUM")
    pa = tc.alloc_tile_pool(name=f"pa{rep}", bufs=1)

    _pending_w = [None]
    w_all = {"q": cst["wq_all"], "k": cst["wk_all"]}
    qk = {}
    vall = {}

    def load_xt(b, tg):
        """Four chunk tiles of 4 cc each covering the tg's 1024 cols. The
        first call also flushes the deferred mask/wq loads right after its
        first group (startup is DMA-arrival-paced; x goes first)."""
        out = []
        for grp in range(4):
            t = pa.tile(
                [P, 4 * TGP], FP16, tag=f"xt{grp}", bufs=2,
                name=f"xt{grp}_{rep}_{b}_{tg}",
            )
            nc.sync.dma_start(
                t[:].rearrange("p (cc w) -> p cc w", cc=4),
                xT[b][
                    P * 4 * grp : P * 4 * (grp + 1), TGP * tg : TGP * (tg + 1)
                ].rearrange("(cc p) w -> p cc w", p=P),
            )
            if _pending_w[0] is not None:
                _pending_w[0]()
                _pending_w[0] = None
            out.append(t)
        return out

    def xsl(xts, cc, c0, w):
        """Moving slice of x chunk cc covering cols [c0, c0+w) of the tg group."""
        t = xts[cc // 4]
        cc0 = cc % 4
        return t[:, TGP * cc0 + c0 : TGP * cc0 + c0 + w]

    def proj_thunks(b):
        """Projection for batch b as a list of thunks, so pass-A attention
        can interleave them into its emission (fills Act-paced PE bubbles)."""
        state = {}
        thunks = []

        def t_alloc():
            va = ab.tile(
                [P, NCC * 2 * VW], FP16, tag="vall", bufs=4, name=f"va_{rep}_{b}"
            )
            vall[b] = va
            nc.gpsimd.memset(
                va[:].rearrange("p (t x) -> p t x", x=VW)[:, :, D : D + 1], 1.0
            )
            for pj in ("q", "k"):
                for h in range(HPC):
                    pool = pa if h == 0 else ab
                    qk[(pj, h, b)] = pool.tile(
                        [P, T], FP16, tag=f"{pj}T{h}", bufs=(2 if h == 0 else 4),
                        name=f"{pj}T{h}_{rep}_{b}",
                    )
            state[0] = load_xt(b, 0)

        thunks.append(t_alloc)

        def t_xt1():
            state[1] = load_xt(b, 1)

        def t_pmm(tg, pj, h):
            xts = state[tg]
            pmm = ps.tile(
                [P, TGP], FP, tag="big2", bufs=3, name=f"pmm_{rep}_{b}_{tg}_{pj}{h}"
            )
            # cc-outer: both 512-halves run back-to-back on the same
            # stationary, halving distinct LDWEIGHTS loads
            for cc in range(NCC):
                for half in range(2):
                    nc.tensor.matmul(
                        pmm[:, XCH * half : XCH * (half + 1)],
                        w_all[pj][:, (HPC * D) * cc + D * h : (HPC * D) * cc + D * (h + 1)],
                        xsl(xts, cc, XCH * half, XCH),
                        start=(cc == 0),
                        stop=(cc == NCC - 1),
                    )
            # RoPE eviction: dst = pmm*C + rot(pmm)*S
            tc_sl = cst["cos_s"][:, TGP * tg : TGP * (tg + 1)]
            ts_sl = cst["sin_s"][:, TGP * tg : TGP * (tg + 1)]
            t1 = pa.tile([P, TGP], FP, tag="t1", bufs=2, name=f"t1_{rep}_{b}_{tg}_{pj}{h}")
            t2 = pa.tile([P, TGP], FP, tag="t2", bufs=2, name=f"t2_{rep}_{b}_{tg}_{pj}{h}")
            nc.vector.tensor_mul(t1[:], pmm[:], tc_sl)
            nc.vector.tensor_mul(t2[0:HALF, :], pmm[HALF:P, :], ts_sl[0:HALF, :])
            nc.vector.tensor_mul(t2[HALF:P, :], pmm[0:HALF, :], ts_sl[HALF:P, :])
            nc.vector.tensor_add(
                qk[(pj, h, b)][:, TGP * tg : TGP * (tg + 1)], t1[:], t2[:]
            )

        def t_v(tg, vg):
            xts = state[tg]
            pv = ps.tile([P, TGP], FP, tag="big2", bufs=3, name=f"pv_{rep}_{b}_{tg}_{vg}")
            for ts4 in range(4):
                for cc in range(NCC):
                    nc.tensor.matmul(
                        pv[:, (HPC * D) * ts4 : (HPC * D) * (ts4 + 1)],
                        xsl(xts, cc, XCH * vg + P * ts4, P),
                        cst["wv_all"][:, (HPC * D) * cc : (HPC * D) * (cc + 1)],
                        start=(cc == 0),
                        stop=(cc == NCC - 1),
                    )
            tch0 = 8 * tg + 4 * vg
            # DVE, not Act: keeps the Act engine's activation table on Exp
            # (each Copy<->Exp switch costs a ~1.3us table load)
            nc.vector.tensor_copy(
                vall[b][:, 2 * VW * tch0 : 2 * VW * (tch0 + 4)].rearrange(
                    "p (t h d) -> p t h d", h=HPC, d=VW
                )[:, :, :, 0:D],
                pv[:].rearrange("p (t h d) -> p t h d", h=HPC, d=D),
            )

        for tg in range(T // TGP):
            if tg == 1:
                thunks.append(t_xt1)
            for pj, h in (("q", 0), ("k", 0), ("q", 1), ("k", 1)):
                thunks.append(partial(t_pmm, tg, pj, h))
            for vg in range(2):
                thunks.append(partial(t_v, tg, vg))
        return thunks

    last_staging = [None]

    def attn(b, h, filler=()):
        filler = list(filler)
        qT = qk[("q", h, b)]
        kT = qk[("k", h, b)]
        va = vall[b]
        seq = [(g, pi) for g in range(NTG) for pi in range(2 * (g + 1))]
        po = {}

        def emit_pv(g, pi, pt):
            poA, poB = po[g]
            for half in range(2):
                i = 2 * pi + half
                jj = i - 4 * g
                for s in range(4):
                    if jj >= 0 and s < jj:
                        continue
                    pair, slot = divmod(s, 2)
                    dst = poA if pair == 0 else poB
                    # start=True clears the whole bank's has_written bits, so
                    # only the FIRST matmul into each pair-tile may set it —
                    # slot 1's first write relies on has_written=0 to store.
                    nc.tensor.matmul(
                        dst[:, VW * slot : VW * (slot + 1)],
                        pt[:, TG * half + P * s : TG * half + P * (s + 1)],
                        va[:, 2 * VW * i + VW * h : 2 * VW * i + VW * (h + 1)],
                        start=(i == 0 and slot == 0),
                        stop=(i == 4 * g + s),
                    )
            if pi == 2 * (g + 1) - 1:
                # group done: normalize + stage
                for pair in range(2):
                    pp = po[g][pair]
                    rc = ab.tile([P, 2], FP, tag="rc", bufs=4, name=f"rc_{rep}_{b}_{h}_{g}_{pair}")
                    ytn = ab.tile(
                        [P, 2 * D], FP16, tag=f"ytn{pair}", bufs=4,
                        name=f"ytn_{rep}_{b}_{h}_{g}_{pair}",
                    )
                    for slot in range(2):
                        nc.vector.reciprocal(
                            rc[:, slot : slot + 1], pp[:, VW * slot + D : VW * slot + D + 1]
                        )
                        nc.vector.tensor_scalar_mul(
                            ytn[:, D * slot : D * (slot + 1)],
                            pp[:, VW * slot : VW * slot + D],
                            rc[:, slot : slot + 1],
                        )
                    n0 = T * b + TG * g + 2 * P * pair
                    last_staging[0] = nc.sync.dma_start(
                        ya_in[h][n0 : n0 + 2 * P, :].rearrange("(s p) d -> p s d", s=2),
                        ytn[:].rearrange("p (s d) -> p s d", s=2),
                    )

        pending = []
        for g, pi in seq:
            if pi == 0:
                po[g] = (
                    ps.tile([P, 2 * VW], FP, tag="poA", bufs=1, name=f"poA_{rep}_{b}_{h}_{g}"),
                    ps.tile([P, 2 * VW], FP, tag="poB", bufs=1, name=f"poB_{rep}_{b}_{h}_{g}"),
                )
            pss = ps.tile([P, 2 * TG], FP, tag="big2", bufs=3, name=f"pss_{rep}_{b}_{h}_{g}_{pi}")
            for half in range(2):
                i = 2 * pi + half
                jj = i - 4 * g
                r = P * jj if jj >= 0 else 0
                nc.tensor.matmul(
                    pss[:, TG * half + r : TG * (half + 1)],
                    kT[:, P * i : P * (i + 1)],
                    qT[:, TG * g + r : TG * (g + 1)],
                    start=True,
                    stop=True,
                )
            pt = ab.tile([P, 2 * TG], FP16, tag="pt", bufs=3, name=f"pt_{rep}_{b}_{h}_{g}_{pi}")
            j0 = 2 * pi - 4 * g
            if j0 >= 2:
                # second diagonal pair: 640 of 1024 columns are dead-masked;
                # exp only the live ranges (two Act instrs beat 640 wasted cols)
                for half in range(2):
                    r = P * (j0 + half)
                    sl_ = slice(TG * half + r, TG * (half + 1))
                    nc.scalar.activation(pt[:, sl_], pss[:, sl_], EXP, scale=SCALE)
            else:
                nc.scalar.activation(pt[:], pss[:], EXP, scale=SCALE)
            for half in range(2):
                i = 2 * pi + half
                jj = i - 4 * g
                if jj >= 0:
                    sl_ = slice(TG * half + P * jj, TG * half + P * (jj + 1))
                    nc.vector.tensor_mul(pt[:, sl_], pt[:, sl_], cst["mask_s"][:])
            if len(pending) >= 2:
                emit_pv(*pending.pop(0))
            if filler:
                filler.pop(0)()
            pending.append((g, pi, pt))
        while pending:
            emit_pv(*pending.pop(0))
        for t in filler:
            t()

    # ---- pass A: projections + h0 attention ---------------------------
    first = rep == 0
    if first:
        # SP queue: first xt chunk group goes ahead of the weight loads
        # (the first matmul chain is DMA-arrival-paced); Act HWDGE queue
        # issues wk/cos/sin/wv in parallel.
        _pending_w[0] = lambda: (
            nc.sync.dma_start(cst["mask_s"][:], cst["maskd"][:]),
            _load_weight(nc, cst["wq_all"], cst["wqT"]),
        )
        nc.scalar.dma_start(
            cst["wk_all"][:].rearrange("p (cc w) -> p cc w", cc=NCC),
            cst["wkT"][:, :].rearrange("(cc p) w -> p cc w", p=P),
        )
        nc.scalar.dma_start(cst["cos_s"][:], cst["cosC"][:])
        nc.scalar.dma_start(cst["sin_s"][:], cst["sinS"][:])
        nc.scalar.dma_start(
            cst["wv_all"][:].rearrange("p (cc w) -> p cc w", cc=NCC),
            cst["wvT"][:, :].rearrange("(cc p) w -> p cc w", p=P),
        )
    for t in proj_thunks(0):
        t()
    for b in range(B):
        filler = proj_thunks(b + 1) if b + 1 < B else []
        attn(b, 0, filler)

    if mode == "full":
        nc.gpsimd.collective_compute(
            "AllToAll",
            mybir.AluOpType.bypass,
            replica_groups=[list(range(N_CORES))],
            ins=[ya_in[0][:]],
            outs=[ya_out[0][:]],
        )

    pa.release()
    wop = tc.alloc_tile_pool(name=f"wo{rep}", bufs=1, side="right")
    ytp = tc.alloc_tile_pool(name=f"ytp{rep}", bufs=1, side="right")
    op3 = tc.alloc_tile_pool(name=f"op3{rep}", bufs=1, side="right")

    # wo load rides the Activation HWDGE queue (SP would head-of-line-block
    # pass-B staging writes behind an 8MB transfer) and is held back until
    # pass-A staging is out, so it doesn't steal startup DMA bandwidth.
    wo_all = wop.tile([P, NCC * C], FP16, tag="wo", name=f"wo_{rep}")
    for wg in range(4):
        wo_dma = nc.scalar.dma_start(
            wo_all[:, NCC * C // 4 * wg : NCC * C // 4 * (wg + 1)].rearrange(
                "p (cc w) -> p cc w", cc=NCC // 4
            ),
            woT[C // 4 * wg : C // 4 * (wg + 1), :].rearrange(
                "(cc p) w -> p cc w", p=P
            ),
        )
        add_dep_helper(wo_dma.ins, last_staging[0].ins, reason="defer wo load")
    yt_all = {
        h: ytp.tile([P, N_CORES * NSLICE], FP16, tag=f"yt{h}", name=f"yt{h}_{rep}")
        for h in range(HPC)
    }

    def load_yt(h, after=None):
        for j in range(N_CORES):
            t = nc.sync.dma_start(
                yt_all[h][:, NSLICE * j : NSLICE * (j + 1)],
                ya_out[h][NSLICE * j : NSLICE * (j + 1), :],
                transpose=True,
            )
            if after is not None:
                # keep the greedy list scheduler from hoisting these into the
                # middle of pass B, where they head-of-line-block the SP DMA
                # queue (staging writes) behind the still-running AllToAll
                add_dep_helper(t.ins, after.ins, reason="defer yt load")

    # ---- pass B: h1 attention (AllToAll(h0) in flight) ----------------
    for b in range(B):
        attn(b, 1)
    # yt(h0) loads go after ALL pass-B staging writes: A2A(h0) is done by
    # now, so these fire immediately without blocking the SP queue.
    load_yt(0, after=last_staging[0])

    if mode == "full":
        nc.gpsimd.collective_compute(
            "AllToAll",
            mybir.AluOpType.bypass,
            replica_groups=[list(range(N_CORES))],
            ins=[ya_in[1][:]],
            outs=[ya_out[1][:]],
        )
    load_yt(1)

    ps.release()
    ps3 = tc.alloc_tile_pool(name=f"ps3{rep}", bufs=1, space="PSUM")

    # ---- output projection, h0-staggered ------------------------------
    jobs = [(jg, nt) for jg in range(C // TG) for nt in range(NSLICE // P)]
    pouts = {}

    def h_mms(idx, hs):
        jg, nt = jobs[idx]
        for j in range(N_CORES):
            ccg = HPC * j + hs
            nc.tensor.matmul(
                pouts[idx][:],
                yt_all[hs][:, NSLICE * j + P * nt : NSLICE * j + P * (nt + 1)],
                wo_all[:, C * ccg + TG * jg : C * ccg + TG * (jg + 1)],
                start=(hs == 0 and j == 0),
                stop=(hs == 1 and j == N_CORES - 1),
            )

    STAG = 8
    for idx in range(len(jobs) + STAG):
        if idx < len(jobs):
            pouts[idx] = ps3.tile([P, TG], FP, tag="pout", bufs=STAG, name=f"pout_{rep}_{idx}")
            h_mms(idx, 0)
        if idx >= STAG:
            k = idx - STAG
            h_mms(k, 1)
            jg, nt = jobs[k]
            ot = op3.tile([P, TG], FP, tag="ot", bufs=2, name=f"ot_{rep}_{k}")
            nc.scalar.copy(ot[:], pouts[k][:])
            nc.sync.dma_start(
                out_rows[P * nt : P * (nt + 1), TG * jg : TG * (jg + 1)], ot[:]
            )
            del pouts[k]

    ps3.release()
    op3.release()
    ytp.release()
    wop.release()
    ab.release()


# ---------------------------------------------------------------------------
# Host-side prep + execution
# ---------------------------------------------------------------------------
def _host_inputs(x, wq, wk, wv, wo):
    xT = np.ascontiguousarray(x.transpose(0, 2, 1)).astype(np.float16)
    woT = np.ascontiguousarray(wo.T).astype(np.float16)

    freqs = 1.0 / (10000.0 ** (np.arange(HALF, dtype=np.float32) / HALF))
    t = np.arange(T, dtype=np.float32)
    ang = freqs[:, None] * t[None, :]  # [64, T]
    cosC = np.concatenate([np.cos(ang), np.cos(ang)], axis=0).astype(np.float16)
    sinS = np.concatenate([-np.sin(ang), np.sin(ang)], axis=0).astype(np.float16)

    # maskd[k, q] = 1.0 iff q >= k
    maskd = np.triu(np.ones((P, P), dtype=np.float16))

    common = dict(xT=xT, woT=woT, cosC=cosC, sinS=sinS, maskd=maskd)
    in_maps = []
    for r in range(N_CORES):
        rows = slice(HPC * D * r, HPC * D * (r + 1))
        in_maps.append(
            dict(
                common,
                wqT=np.ascontiguousarray(wq[rows, :].T).astype(np.float16),
                wkT=np.ascontiguousarray(wk[rows, :].T).astype(np.float16),
                wvT=np.ascontiguousarray(wv[rows, :].T).astype(np.float16),
            )
        )
    return in_maps


_CACHED = {}


def _get_program(reps=1):
    if reps not in _CACHED:
        _CACHED[reps] = build_program(reps)[0]
    return _CACHED[reps]


def kernel(x, wq, wk, wv, wo):
    nc = _get_program(1)
    in_maps = _host_inputs(
        np.asarray(x, dtype=np.float32),
        np.asarray(wq, dtype=np.float32),
        np.asarray(wk, dtype=np.float32),
        np.asarray(wv, dtype=np.float32),
        np.asarray(wo, dtype=np.float32),
    )
    res = run_bass_kernel_spmd(nc, in_maps, list(range(N_CORES)))
    out = np.concatenate([res.results[r]["out_rows"] for r in range(N_CORES)], axis=0)
    return out.reshape(B, T, C)
